# revision 1
# baseline (speedup 1.0000x reference)
"""Multi-head attention (B=4, S=2048, D=1024, H=16, causal) on 8 TRN2 cores.

Sharding: core c -> (batch b = c//2, head-group g = c%2 of 8 heads).
Each core computes projections for its 8 heads (column-split Wq/Wk/Wv),
flash-style causal attention, and a partial output projection (row-split Wo).
Host unshard sums the two partials per batch and adds bo.

Device layouts: activations transposed (qwT/kwT: [dout, seq]) so attention
matmuls need no on-device transposes; vw natural [seq, dout] with a ones
column per head so the PV matmul (M=65) emits softmax denominators for free.
All heavy matmuls run as float32r (full-rate fp32 path on the TRN2 PE).
"""

from contextlib import ExitStack

import numpy as np

import concourse.bass as bass
import concourse.tile as tile
from concourse import bacc, mybir
from concourse.bass_utils import run_bass_kernel_spmd

F32 = mybir.dt.float32
F32R = mybir.dt.float32r
EXP = mybir.ActivationFunctionType.Exp
COPY = mybir.ActivationFunctionType.Copy

B, S, D, H = 4, 2048, 1024, 16
HD = D // H          # 64
DL = D // 2          # 512 local douts per core
NT = DL // 128       # 4 dout tiles
NR = S // 128        # 16 row tiles
NQ = S // 512        # 4 query chunks
NDIN = D // 128      # 8 din tiles

# tunables
PACK = True          # row-pack 2 heads (K=64) in scores matmuls
GROUP = 4            # j-blocks per scores/PV sub-batch (PE mode-switch amortization)
BC_F32R = False      # use f32r for the recip-broadcast matmul (exactness TBD on hw)


def _r(ap):
    return ap.bitcast(F32R)


def round_f32r(x):
    """Round fp32 array to the fp32r grid (11 mantissa bits, RNE at bit 12)."""
    u = np.ascontiguousarray(x, np.float32).view(np.uint32)
    r = (u + 0x7FF + ((u >> 12) & 1)) & np.uint32(0xFFFFF000)
    return r.view(np.float32)


def build_nc():
    nc = bacc.Bacc("TRN2", target_bir_lowering=False, debug=False, num_devices=8)

    qT = nc.dram_tensor("qT", [D, S], F32R, kind="ExternalInput").ap()
    kT = nc.dram_tensor("kT", [D, S], F32R, kind="ExternalInput").ap()
    vT = nc.dram_tensor("vT", [D, S], F32R, kind="ExternalInput").ap()
    Wq_s = nc.dram_tensor("Wq_s", [D, DL], F32R, kind="ExternalInput").ap()
    Wk_s = nc.dram_tensor("Wk_s", [D, DL], F32R, kind="ExternalInput").ap()
    Wv_s = nc.dram_tensor("Wv_s", [D, DL], F32R, kind="ExternalInput").ap()
    Wo_s = nc.dram_tensor("Wo_s", [DL, D], F32R, kind="ExternalInput").ap()
    bq_s = nc.dram_tensor("bq_s", [DL, 1], F32, kind="ExternalInput").ap()
    bk_s = nc.dram_tensor("bk_s", [DL, 1], F32, kind="ExternalInput").ap()
    bv_bc = nc.dram_tensor("bv_bc", [128, DL], F32, kind="ExternalInput").ap()
    E_in = nc.dram_tensor("E_in", [8, DL], F32, kind="ExternalInput").ap()
    out_p = nc.dram_tensor("out_partial", [S, D], F32, kind="ExternalOutput").ap()

    with tile.TileContext(nc) as tc, ExitStack() as ctx:
        # ---------------- persistent SBUF ----------------
        keep = ctx.enter_context(tc.tile_pool(name="keep", bufs=1))
        qwT = [keep.tile([128, S], F32R, tag=f"qwT{t}", name=f"qwT{t}") for t in range(NT)]
        kwT = [keep.tile([128, S], F32R, tag=f"kwT{t}", name=f"kwT{t}") for t in range(NT)]
        vw = [keep.tile([128, 8 * 65], F32R, tag=f"vw{r}", name=f"vw{r}") for r in range(NR)]
        sums = keep.tile([8, S], F32, tag="sums")
        recip = keep.tile([8, S], F32, tag="recip")
        bias_q = keep.tile([128, NT], F32, tag="bias_q")  # col t = bq tile t
        bias_k = keep.tile([128, NT], F32, tag="bias_k")
        bv_sb = keep.tile([128, DL], F32, tag="bv_sb")
        E_sb = keep.tile([8, DL], F32, tag="E_sb")

        for t in range(NT):
            nc.sync.dma_start(bias_q[:, t:t + 1], bq_s[128 * t:128 * (t + 1), :])
            nc.sync.dma_start(bias_k[:, t:t + 1], bk_s[128 * t:128 * (t + 1), :])
        nc.sync.dma_start(bv_sb[:], bv_bc)
        nc.sync.dma_start(E_sb[:], E_in)
        bv3 = bv_sb[:].rearrange("p (a b) -> p a b", b=1)
        for r in range(NR):
            ones_ap = vw[r][:].rearrange("p (h e) -> p h e", e=65)[:, :, 64:65]
            nc.scalar.activation(ones_ap, bv3[:, 0:8, :], COPY, bias=1.0, scale=0.0)

        # ---------------- projections ----------------
        with tc.tile_pool(name="wt", bufs=1) as wtp, \
             tc.tile_pool(name="slab", bufs=1) as slp, \
             tc.tile_pool(name="pps", bufs=3, space="PSUM") as pps:

            def load_w(W):
                w_sb = []
                for dn in range(NDIN):
                    w = wtp.tile([128, DL], F32R, tag=f"w{dn}")
                    nc.sync.dma_start(w[:], W[128 * dn:128 * (dn + 1), :])
                    w_sb.append(w)
                return w_sb

            def load_slab(xT):
                sl = []
                for dn in range(NDIN):
                    s_ = slp.tile([128, S], F32R, tag=f"sl{dn}")
                    nc.sync.dma_start(s_[:], xT[128 * dn:128 * (dn + 1), :])
                    sl.append(s_)
                return sl

            def proj_T(xT, W, bias_t, dst):
                # dst[t] [128, S] = tile t of (x @ W).T + bias (douts on partitions)
                w_sb = load_w(W)
                sl = load_slab(xT)
                for t in range(NT):
                    for rc in range(NQ):
                        ps = pps.tile([128, 512], F32, tag="pp")
                        for dn in range(NDIN):
                            nc.tensor.matmul(
                                ps[:],
                                w_sb[dn][:, 128 * t:128 * (t + 1)],
                                sl[dn][:, 512 * rc:512 * (rc + 1)],
                                start=(dn == 0), stop=(dn == NDIN - 1))
                        nc.vector.tensor_scalar_add(
                            dst[t][:, 512 * rc:512 * (rc + 1)],
                            ps[:], bias_t[:, t:t + 1])

            proj_T(qT, Wq_s, bias_q, qwT)
            proj_T(kT, Wk_s, bias_k, kwT)

            # vw natural: [row, dout] with per-head ones column
            w_sb = load_w(Wv_s)
            sl = load_slab(vT)
            for r in range(NR):
                ps = pps.tile([128, 512], F32, tag="pp")
                for dn in range(NDIN):
                    nc.tensor.matmul(
                        ps[:],
                        sl[dn][:, 128 * r:128 * (r + 1)],
                        w_sb[dn][:],
                        start=(dn == 0), stop=(dn == NDIN - 1))
                dst3 = vw[r][:].rearrange("p (h e) -> p h e", e=65)[:, :, 0:64]
                nc.vector.tensor_add(
                    dst3, ps[:].rearrange("p (h e) -> p h e", e=64),
                    bv_sb[:].rearrange("p (h e) -> p h e", e=64))

        # ---------------- attention + normalize + output projection ----------------
        with tc.tile_pool(name="atnp", bufs=1) as anp:
            atn = [anp.tile([128, S], F32R, tag=f"atn{t}", name=f"atn{t}") for t in range(NT)]

            with tc.tile_pool(name="probs", bufs=2) as prp, \
                 tc.tile_pool(name="scps", bufs=3, space="PSUM") as scp, \
                 tc.tile_pool(name="atps", bufs=1, space="PSUM") as atp:

                for p in range(NT):  # head pair p -> local heads (2p, 2p+1)
                    for qc in range(NQ):
                        jmax = 4 * qc + 3
                        atA = atp.tile([65, 512], F32, tag="atA")
                        atB = atp.tile([65, 512], F32, tag="atB")
                        for j0 in range(0, jmax + 1, GROUP):
                            js = range(j0, min(j0 + GROUP, jmax + 1))
                            pr = {}
                            for j in js:
                                off = max(0, 128 * j - 512 * qc)
                                qs = slice(512 * qc + off, 512 * (qc + 1))
                                sA = scp.tile([128, 512], F32, tag="sA")
                                sB = scp.tile([128, 512], F32, tag="sB")
                                tpA = (0, 0) if PACK else None
                                tpB = (64, 0) if PACK else None
                                nc.tensor.matmul(
                                    sA[:, off:512],
                                    kwT[p][0:64, 128 * j:128 * (j + 1)],
                                    qwT[p][0:64, qs],
                                    start=True, stop=True, tile_position=tpA)
                                nc.tensor.matmul(
                                    sB[:, off:512],
                                    kwT[p][64:128, 128 * j:128 * (j + 1)],
                                    qwT[p][64:128, qs],
                                    start=True, stop=True, tile_position=tpB)
                                pA = prp.tile([128, 512], F32R, tag=f"pA{j % GROUP}")
                                pB = prp.tile([128, 512], F32R, tag=f"pB{j % GROUP}")
                                nc.scalar.activation(pA[:, off:512], sA[:, off:512],
                                                     EXP, scale=1.0 / 8.0)
                                nc.scalar.activation(pB[:, off:512], sB[:, off:512],
                                                     EXP, scale=1.0 / 8.0)
                                if 128 * j >= 512 * qc:  # diagonal block
                                    for pp_ in (pA, pB):
                                        nc.gpsimd.affine_select(
                                            out=pp_[:, off:off + 128],
                                            in_=pp_[:, off:off + 128],
                                            channel_multiplier=-1,
                                            pattern=[[1, 128]], base=0,
                                            compare_op=mybir.AluOpType.is_ge,
                                            fill=0.0)
                                pr[j] = (pA, pB, off)
                            for j in js:
                                pA, pB, off = pr[j]
                                nc.tensor.matmul(
                                    atA[0:65, off:512],
                                    vw[j][:, 65 * 2 * p:65 * 2 * p + 65],
                                    pA[:, off:512],
                                    start=(j == 0), stop=(j == jmax))
                                nc.tensor.matmul(
                                    atB[0:65, off:512],
                                    vw[j][:, 65 * (2 * p + 1):65 * (2 * p + 1) + 65],
                                    pB[:, off:512],
                                    start=(j == 0), stop=(j == jmax))
                        qf = slice(512 * qc, 512 * (qc + 1))
                        nc.vector.tensor_copy(atn[p][0:64, qf], atA[0:64, :])
                        nc.vector.tensor_copy(atn[p][64:128, qf], atB[0:64, :])
                        stgA = prp.tile([1, 512], F32, tag="stgA")
                        stgB = prp.tile([1, 512], F32, tag="stgB")
                        nc.scalar.activation(stgA[:], atA[64:65, :], COPY)
                        nc.scalar.activation(stgB[:], atB[64:65, :], COPY)
                        nc.sync.dma_start(sums[2 * p:2 * p + 1, qf], stgA[:])
                        nc.sync.dma_start(sums[2 * p + 1:2 * p + 2, qf], stgB[:])

            nc.vector.reciprocal(recip[:], sums[:])

            with tc.tile_pool(name="wo", bufs=1) as wop, \
                 tc.tile_pool(name="osb", bufs=3) as osp, \
                 tc.tile_pool(name="bcps", bufs=2, space="PSUM") as bcp, \
                 tc.tile_pool(name="ops", bufs=2, space="PSUM") as opp:

                wo_sb = []
                for t in range(NT):
                    w = wop.tile([128, D], F32R, tag=f"wo{t}")
                    nc.sync.dma_start(w[:], Wo_s[128 * t:128 * (t + 1), :])
                    wo_sb.append(w)

                for t in range(NT):
                    for qc in range(NQ):
                        qf = slice(512 * qc, 512 * (qc + 1))
                        bc = bcp.tile([128, 512], F32, tag="bc")
                        lhs = E_sb[:, 128 * t:128 * (t + 1)]
                        rhs = recip[:, qf]
                        if BC_F32R:
                            lhs, rhs = _r(lhs), _r(rhs)
                        nc.tensor.matmul(bc[:], lhs, rhs, start=True, stop=True)
                        nc.vector.tensor_mul(atn[t][:, qf], atn[t][:, qf].bitcast(F32), bc[:])

                for rt in range(NR):
                    po = opp.tile([128, D], F32, tag="po")
                    for nch in range(2):
                        for t in range(NT):
                            nc.tensor.matmul(
                                po[:, 512 * nch:512 * (nch + 1)],
                                atn[t][:, 128 * rt:128 * (rt + 1)],
                                wo_sb[t][:, 512 * nch:512 * (nch + 1)],
                                start=(t == 0), stop=(t == NT - 1))
                    ob = osp.tile([128, D], F32, tag="ob")
                    nc.vector.tensor_copy(ob[:], po[:])
                    nc.sync.dma_start(out_p[128 * rt:128 * (rt + 1), :], ob[:])

    nc.compile()
    return nc


_NC_CACHE = {}


def get_nc():
    if "nc" not in _NC_CACHE:
        _NC_CACHE["nc"] = build_nc()
    return _NC_CACHE["nc"]


def make_in_maps(q, k, v, Wq, bq, Wk, bk, Wv, bv, Wo):
    """Host-side shard prep. Returns list of 8 per-core input dicts."""
    f = np.float32
    q = np.asarray(q, f)
    k = np.asarray(k, f)
    v = np.asarray(v, f)
    Wq, bq = np.asarray(Wq, f), np.asarray(bq, f)
    Wk, bk = np.asarray(Wk, f), np.asarray(bk, f)
    Wv, bv = np.asarray(Wv, f), np.asarray(bv, f)
    Wo = np.asarray(Wo, f)
    E = np.zeros((8, DL), f)
    for h in range(8):
        E[h, 64 * h:64 * (h + 1)] = 1.0
    in_maps = []
    for c in range(8):
        b, g = c // 2, c % 2
        cs = slice(DL * g, DL * (g + 1))
        in_maps.append(dict(
            qT=round_f32r(q[b].T),
            kT=round_f32r(k[b].T),
            vT=round_f32r(v[b].T),
            Wq_s=round_f32r(Wq[:, cs]),
            Wk_s=round_f32r(Wk[:, cs]),
            Wv_s=round_f32r(Wv[:, cs]),
            Wo_s=round_f32r(Wo[cs, :]),
            bq_s=np.ascontiguousarray(bq[cs]).reshape(DL, 1),
            bk_s=np.ascontiguousarray(bk[cs]).reshape(DL, 1),
            bv_bc=np.tile(bv[cs][None, :], (128, 1)),
            E_in=E,
        ))
    return in_maps


def unshard(results, bo):
    bo = np.asarray(bo, np.float32)
    out = np.empty((B, S, D), np.float32)
    for b in range(B):
        out[b] = (results[2 * b]["out_partial"]
                  + results[2 * b + 1]["out_partial"] + bo)
    return out


def kernel(q, k, v, mask, Wq, bq, Wk, bk, Wv, bv, Wo, bo, **_unused):
    nc = get_nc()
    in_maps = make_in_maps(q, k, v, Wq, bq, Wk, bk, Wv, bv, Wo)
    res = run_bass_kernel_spmd(nc, in_maps, core_ids=list(range(8))).results
    return unshard(res, bo)



# revision 6
# speedup vs baseline: 2.0057x; 2.0057x over previous
"""Multi-head attention (B=4, S=2048, D=1024, H=16, causal) on 8 TRN2 cores.

Sharding: core c -> (batch b = c//2, head-group g = c%2 of 8 heads).
Each core computes projections for its 8 heads (column-split Wq/Wk/Wv),
flash-style causal attention, and a partial output projection (row-split Wo).
Host unshard sums the two partials per batch and adds bo.

v2 (all-bf16, fully software-pipelined single pass):
- Everything bf16 on the PE (error budget allows: score errors shrink 8x
  under the 1/sqrt(HD) softmax scale). Halves DMA + SBUF, removes the
  fp32r N<256 4x-cycle penalty, enables FWL weight loads.
- qc-outer schedule: per query chunk, all head pairs run scores->exp->PV
  with a 2-step lag (PV of j-2 issues after scores of j), and projection /
  output-projection matmuls are issued between attention steps as PE
  filler so the PE never idles and HAM stays at 2.4 GHz.
- A+B exp fused into one [128,1024] ACTIVATE per j-block.
- Attention output is written back into the qwT tile (queries for chunk qc
  are dead after their scores), saving 16KB/partition of SBUF.
- Denominators ride the PV matmul as a per-head ones column (M=65).
"""

from collections import deque
from contextlib import ExitStack

import ml_dtypes
import numpy as np

import concourse.bass as bass
import concourse.tile as tile
from concourse import bacc, mybir
from concourse.bass_utils import run_bass_kernel_spmd

F32 = mybir.dt.float32
BF = mybir.dt.bfloat16
EXP = mybir.ActivationFunctionType.Exp
COPY = mybir.ActivationFunctionType.Copy
BF_NP = ml_dtypes.bfloat16

B, S, D, H = 4, 2048, 1024, 16
HD = D // H          # 64
DL = D // 2          # 512 local douts per core
NT = DL // 128       # 4 dout tiles / head pairs
NR = S // 128        # 16 key row tiles
NQ = S // 512        # 4 query chunks
NDIN = D // 128      # 8 din tiles
LAG = 2              # scores(j) -> PV(j-LAG) software pipeline depth


def build_nc():
    nc = bacc.Bacc("TRN2", target_bir_lowering=False, debug=False, num_devices=8)

    qT = nc.dram_tensor("qT", [D, S], BF, kind="ExternalInput").ap()
    kT = nc.dram_tensor("kT", [D, S], BF, kind="ExternalInput").ap()
    vT = nc.dram_tensor("vT", [D, S], BF, kind="ExternalInput").ap()
    Wq_s = nc.dram_tensor("Wq_s", [D, DL], BF, kind="ExternalInput").ap()
    Wk_s = nc.dram_tensor("Wk_s", [D, DL], BF, kind="ExternalInput").ap()
    Wv_s = nc.dram_tensor("Wv_s", [D, DL], BF, kind="ExternalInput").ap()
    Wo_s = nc.dram_tensor("Wo_s", [DL, D], BF, kind="ExternalInput").ap()
    bq_s = nc.dram_tensor("bq_s", [DL, 1], F32, kind="ExternalInput").ap()
    bk_s = nc.dram_tensor("bk_s", [DL, 1], F32, kind="ExternalInput").ap()
    bv_bc = nc.dram_tensor("bv_bc", [128, DL], F32, kind="ExternalInput").ap()
    E_in = nc.dram_tensor("E_in", [8, DL], BF, kind="ExternalInput").ap()
    out_p = nc.dram_tensor("out_partial", [S, D], F32, kind="ExternalOutput").ap()

    with tile.TileContext(nc) as tc, ExitStack() as ctx:
        keep = ctx.enter_context(tc.tile_pool(name="keep", bufs=1))
        qslp = ctx.enter_context(tc.tile_pool(name="qsl", bufs=2))
        kslp = ctx.enter_context(tc.tile_pool(name="ksl", bufs=2))
        vslp = ctx.enter_context(tc.tile_pool(name="vsl", bufs=2))
        wtp = ctx.enter_context(tc.tile_pool(name="wt", bufs=1))
        prp = ctx.enter_context(tc.tile_pool(name="probs", bufs=1))
        stgp = ctx.enter_context(tc.tile_pool(name="stg", bufs=2))
        osp = ctx.enter_context(tc.tile_pool(name="osb", bufs=3))
        scp = ctx.enter_context(tc.tile_pool(name="scps", bufs=2, space="PSUM"))
        atp = ctx.enter_context(tc.tile_pool(name="atps", bufs=1, space="PSUM"))
        mip = ctx.enter_context(tc.tile_pool(name="mips", bufs=2, space="PSUM"))

        # ---------------- persistent SBUF ----------------
        # hw[t]: Q^T for head pair t during scores, then overwritten per qc
        # chunk with the (unnormalized) attention output of pair t.
        hw = [keep.tile([128, S], BF, tag=f"hw{t}", name=f"hw{t}") for t in range(NT)]
        kwT = [keep.tile([128, S], BF, tag=f"kwT{t}", name=f"kwT{t}") for t in range(NT)]
        vw = [keep.tile([128, 8 * 65], BF, tag=f"vw{r}", name=f"vw{r}") for r in range(NR)]
        sums = keep.tile([8, S], F32, tag="sums")
        recip = keep.tile([8, S], BF, tag="recip")
        bias_q = keep.tile([128, NT], F32, tag="bias_q")
        bias_k = keep.tile([128, NT], F32, tag="bias_k")
        bv_sb = keep.tile([128, DL], F32, tag="bv_sb")
        E_sb = keep.tile([8, DL], BF, tag="E_sb")

        # small/constant loads (gpsimd DMA queue)
        for t in range(NT):
            nc.gpsimd.dma_start(bias_q[:, t:t + 1], bq_s[128 * t:128 * (t + 1), :])
            nc.gpsimd.dma_start(bias_k[:, t:t + 1], bk_s[128 * t:128 * (t + 1), :])
        nc.gpsimd.dma_start(bv_sb[:], bv_bc)
        nc.gpsimd.dma_start(E_sb[:], E_in)

        # per-head ones column in vw (PV emits softmax denominators for free)
        bv3 = bv_sb[:].rearrange("p (a b) -> p a b", b=1)
        for r in range(NR):
            ones_ap = vw[r][:].rearrange("p (h e) -> p h e", e=65)[:, :, 64:65]
            nc.scalar.activation(ones_ap, bv3[:, 0:8, :], COPY, bias=1.0, scale=0.0)

        # projection weights (gpsimd DMA queue)
        wq_sb, wk_sb, wv_sb = [], [], []
        for pfx, dst, W in (("wq", wq_sb, Wq_s), ("wk", wk_sb, Wk_s),
                            ("wv", wv_sb, Wv_s)):
            for dn in range(NDIN):
                w = wtp.tile([128, DL], BF, tag=f"{pfx}{dn}")
                nc.gpsimd.dma_start(w[:], W[128 * dn:128 * (dn + 1), :])
                dst.append(w)
        wo_sb = []
        for t in range(NT):
            w = wtp.tile([128, D], BF, tag=f"wo{t}")
            nc.gpsimd.dma_start(w[:], Wo_s[128 * t:128 * (t + 1), :])
            wo_sb.append(w)

        # ---------------- chunked slab loads (sync DMA queue) ----------------
        qsl = {}  # qsl[qc][dn] -> [128, 512] bf16 tile of qT
        ksl = {}
        vsl = {}

        def load_chunk(store, pool, src, c, pfx):
            tiles = []
            for dn in range(NDIN):
                t_ = pool.tile([128, 512], BF, tag=f"{pfx}{dn}")
                nc.sync.dma_start(t_[:], src[128 * dn:128 * (dn + 1), 512 * c:512 * (c + 1)])
                tiles.append(t_)
            store[c] = tiles

        # ---------------- filler generators ----------------
        proj_gens = deque()
        out_gens = deque()

        def fill(n):
            done = 0
            while done < n:
                q = proj_gens if proj_gens else out_gens
                if not q:
                    return
                try:
                    next(q[0])
                    done += 1
                except StopIteration:
                    q.popleft()

        def drain_proj():
            while proj_gens:
                try:
                    next(proj_gens[0])
                except StopIteration:
                    proj_gens.popleft()

        def drain_all():
            drain_proj()
            while out_gens:
                try:
                    next(out_gens[0])
                except StopIteration:
                    out_gens.popleft()

        def projQ_gen(qc):
            for t in range(NT):
                ps = mip.tile([128, 512], F32, tag="mx")
                for dn in range(NDIN):
                    nc.tensor.matmul(
                        ps[:], wq_sb[dn][:, 128 * t:128 * (t + 1)], qsl[qc][dn][:],
                        start=(dn == 0), stop=(dn == NDIN - 1))
                    if dn % 2 == 1:
                        yield
                nc.vector.tensor_scalar_add(
                    hw[t][:, 512 * qc:512 * (qc + 1)], ps[:], bias_q[:, t:t + 1])

        def projK_gen(rc):
            for t in range(NT):
                ps = mip.tile([128, 512], F32, tag="mx")
                for dn in range(NDIN):
                    nc.tensor.matmul(
                        ps[:], wk_sb[dn][:, 128 * t:128 * (t + 1)], ksl[rc][dn][:],
                        start=(dn == 0), stop=(dn == NDIN - 1))
                    if dn % 2 == 1:
                        yield
                nc.vector.tensor_scalar_add(
                    kwT[t][:, 512 * rc:512 * (rc + 1)], ps[:], bias_k[:, t:t + 1])

        def projV_gen(g):
            for r in range(4 * g, 4 * g + 4):
                ps = mip.tile([128, 512], F32, tag="mx")
                for dn in range(NDIN):
                    nc.tensor.matmul(
                        ps[:], vsl[g][dn][:, 128 * (r - 4 * g):128 * (r - 4 * g + 1)],
                        wv_sb[dn][:],
                        start=(dn == 0), stop=(dn == NDIN - 1))
                    if dn % 2 == 1:
                        yield
                dst3 = vw[r][:].rearrange("p (h e) -> p h e", e=65)[:, :, 0:64]
                nc.vector.tensor_add(
                    dst3, ps[:].rearrange("p (h e) -> p h e", e=64),
                    bv_sb[:].rearrange("p (h e) -> p h e", e=64))

        def outproj_gen(qc):
            for rt in range(4 * qc, 4 * qc + 4):
                for nch in range(2):
                    po = mip.tile([128, 512], F32, tag="mx")
                    for t in range(NT):
                        nc.tensor.matmul(
                            po[:], hw[t][:, 128 * rt:128 * (rt + 1)],
                            wo_sb[t][:, 512 * nch:512 * (nch + 1)],
                            start=(t == 0), stop=(t == NT - 1))
                        if t % 2 == 1:
                            yield
                    ob = osp.tile([128, 512], F32, tag="ob")
                    nc.vector.tensor_copy(ob[:], po[:])
                    nc.scalar.dma_start(
                        out_p[128 * rt:128 * (rt + 1), 512 * nch:512 * (nch + 1)], ob[:])

        # ---------------- attention ----------------
        def attention(p, qc):
            jmax = 4 * qc + 3
            atA = atp.tile([65, 512], F32, tag="atA")
            atB = atp.tile([65, 512], F32, tag="atB")
            pend = {}
            for step in range(jmax + 1 + LAG):
                if step <= jmax:
                    j = step
                    off = max(0, 128 * j - 512 * qc)
                    qs = slice(512 * qc + off, 512 * (qc + 1))
                    sAB = scp.tile([128, 1024], F32, tag="sAB")
                    nc.tensor.matmul(
                        sAB[:, off:512],
                        kwT[p][0:64, 128 * j:128 * (j + 1)], hw[p][0:64, qs],
                        start=True, stop=True, tile_position=(0, 0))
                    nc.tensor.matmul(
                        sAB[:, 512 + off:1024],
                        kwT[p][64:128, 128 * j:128 * (j + 1)], hw[p][64:128, qs],
                        start=True, stop=True, tile_position=(64, 0))
                    pAB = prp.tile([128, 1024], BF, tag=f"p{j % 6}")
                    if off == 0:
                        nc.scalar.activation(pAB[:], sAB[:], EXP, scale=0.125)
                    else:
                        nc.scalar.activation(pAB[:, off:512], sAB[:, off:512],
                                             EXP, scale=0.125)
                        nc.scalar.activation(pAB[:, 512 + off:1024],
                                             sAB[:, 512 + off:1024], EXP, scale=0.125)
                    if 128 * j >= 512 * qc:  # diagonal block: causal mask
                        for cb in (off, 512 + off):
                            nc.gpsimd.affine_select(
                                out=pAB[:, cb:cb + 128], in_=pAB[:, cb:cb + 128],
                                channel_multiplier=-1, pattern=[[1, 128]], base=0,
                                compare_op=mybir.AluOpType.is_ge, fill=0.0)
                    pend[j] = (pAB, off)
                jj = step - LAG
                if 0 <= jj:
                    pAB, off = pend.pop(jj)
                    nc.tensor.matmul(
                        atA[0:65, off:512],
                        vw[jj][:, 65 * 2 * p:65 * 2 * p + 65], pAB[:, off:512],
                        start=(jj == 0), stop=(jj == jmax))
                    nc.tensor.matmul(
                        atB[0:65, off:512],
                        vw[jj][:, 65 * (2 * p + 1):65 * (2 * p + 1) + 65],
                        pAB[:, 512 + off:1024],
                        start=(jj == 0), stop=(jj == jmax))
                fill(1)
            # epilogue: write attention output over the dead Q columns + stage
            # the denominators (PSUM row 64) out to sums via SBUF.
            qf = slice(512 * qc, 512 * (qc + 1))
            nc.vector.tensor_copy(hw[p][0:64, qf], atA[0:64, :])
            nc.vector.tensor_copy(hw[p][64:128, qf], atB[0:64, :])
            stgA = stgp.tile([1, 512], F32, tag="stgA")
            stgB = stgp.tile([1, 512], F32, tag="stgB")
            nc.vector.tensor_copy(stgA[:], atA[64:65, :])
            nc.vector.tensor_copy(stgB[:], atB[64:65, :])
            nc.gpsimd.dma_start(sums[2 * p:2 * p + 1, qf], stgA[:])
            nc.gpsimd.dma_start(sums[2 * p + 1:2 * p + 2, qf], stgB[:])

        # ---------------- schedule ----------------
        load_chunk(qsl, qslp, qT, 0, "q")
        load_chunk(ksl, kslp, kT, 0, "k")
        load_chunk(vsl, vslp, vT, 0, "v")

        proj_gens.append(projQ_gen(0))
        proj_gens.append(projK_gen(0))
        proj_gens.append(projV_gen(0))
        drain_proj()

        for qc in range(NQ):
            if qc + 1 < NQ:
                load_chunk(qsl, qslp, qT, qc + 1, "q")
                load_chunk(ksl, kslp, kT, qc + 1, "k")
                load_chunk(vsl, vslp, vT, qc + 1, "v")
                proj_gens.append(projQ_gen(qc + 1))
                proj_gens.append(projK_gen(qc + 1))
                proj_gens.append(projV_gen(qc + 1))
            for p in range(NT):
                attention(p, qc)
            drain_proj()  # next chunk's projections must complete before use
            qf = slice(512 * qc, 512 * (qc + 1))
            with nc.allow_low_precision(reason="bf16 recip feeds bf16 matmul"):
                nc.vector.reciprocal(recip[:, qf], sums[:, qf])
            for t in range(NT):
                bc = mip.tile([128, 512], F32, tag="mx")
                nc.tensor.matmul(bc[:], E_sb[:, 128 * t:128 * (t + 1)],
                                 recip[:, qf], start=True, stop=True)
                nc.vector.tensor_mul(hw[t][:, qf], hw[t][:, qf], bc[:])
            out_gens.append(outproj_gen(qc))
        drain_all()

    nc.compile()
    return nc


_NC_CACHE = {}


def get_nc():
    if "nc" not in _NC_CACHE:
        _NC_CACHE["nc"] = build_nc()
    return _NC_CACHE["nc"]


def _bf(x):
    return np.ascontiguousarray(np.asarray(x, np.float32)).astype(BF_NP)


def make_in_maps(q, k, v, Wq, bq, Wk, bk, Wv, bv, Wo):
    """Host-side shard prep. Returns list of 8 per-core input dicts."""
    f = np.float32
    q = np.asarray(q, f)
    k = np.asarray(k, f)
    v = np.asarray(v, f)
    Wq, bq = np.asarray(Wq, f), np.asarray(bq, f)
    Wk, bk = np.asarray(Wk, f), np.asarray(bk, f)
    Wv, bv = np.asarray(Wv, f), np.asarray(bv, f)
    Wo = np.asarray(Wo, f)
    E = np.zeros((8, DL), f)
    for h in range(8):
        E[h, 64 * h:64 * (h + 1)] = 1.0
    in_maps = []
    for c in range(8):
        b, g = c // 2, c % 2
        cs = slice(DL * g, DL * (g + 1))
        in_maps.append(dict(
            qT=_bf(q[b].T),
            kT=_bf(k[b].T),
            vT=_bf(v[b].T),
            Wq_s=_bf(Wq[:, cs]),
            Wk_s=_bf(Wk[:, cs]),
            Wv_s=_bf(Wv[:, cs]),
            Wo_s=_bf(Wo[cs, :]),
            bq_s=np.ascontiguousarray(bq[cs]).reshape(DL, 1),
            bk_s=np.ascontiguousarray(bk[cs]).reshape(DL, 1),
            bv_bc=np.tile(bv[cs][None, :], (128, 1)).astype(f),
            E_in=E.astype(BF_NP),
        ))
    return in_maps


def unshard(results, bo):
    bo = np.asarray(bo, np.float32)
    out = np.empty((B, S, D), np.float32)
    for b in range(B):
        out[b] = (results[2 * b]["out_partial"]
                  + results[2 * b + 1]["out_partial"] + bo)
    return out


def kernel(q, k, v, mask, Wq, bq, Wk, bk, Wv, bv, Wo, bo, **_unused):
    nc = get_nc()
    in_maps = make_in_maps(q, k, v, Wq, bq, Wk, bk, Wv, bv, Wo)
    res = run_bass_kernel_spmd(nc, in_maps, core_ids=list(range(8))).results
    return unshard(res, bo)
